# revision 6
# baseline (speedup 1.0000x reference)
"""DeepseekV2-Lite MLA attention on 8 Trainium2 NeuronCores.

Sharding: DP2 (batch) x TP4 (4 heads/core). Core c handles batch c//4,
heads [4*(c%4), 4*(c%4)+4). Each core computes a partial output (its
heads' contribution to the o-projection); the host sums 4 partials per
batch.

Device algorithm per core (T=2048 tokens, H=4 heads), all big matmuls in
float32r (TF32-like, full PE rate at N>=256):
  Phase 1 (per 512-token group): kv_a projection (c + shared k_pe) in
    token-major layout, RMSNorm over free axis, RoPE on k_pe, PE-transpose
    c_ln -> c_lnT, kv_b decompression to k_nopeT (head-major transposed)
    and v (token-major).
  Phase 2 (per 512-q group): q projections (nope directly transposed,
    rope token-major then RoPE + PE-transpose), then flash-style
    attention with k-major score tiles: sT = kT q, p = exp(sT) (no
    max subtraction -- |scores| <= ~5), causal mask via affine_select,
    l = ones^T p via matmul, outT = v^T p accumulated in PSUM, out/l.
  Phase 3: o-projection from outT (DRAM-staged) with per-core o_w slice.
"""

import math
import numpy as np

import concourse.bacc as bacc
import concourse.mybir as mybir
import concourse.tile as tile
from concourse.bass_utils import run_bass_kernel_spmd

# ---- problem constants (hardcoded per contest contract) ----
B, S, HID = 2, 2048, 2048
NH, D_NOPE, D_ROPE, D_V = 16, 128, 64, 128
D_Q = D_NOPE + D_ROPE
KV_RANK = 512
EPS = 1e-6
MAX_POS = 8192
BASE = 10000.0
FACTOR = 40.0
ORIG_MAX = 4096
BETA_FAST, BETA_SLOW = 32, 1
MSCALE, MSCALE_ALL = 0.707, 0.707

H = 4            # heads per core
NG = 4           # 512-token groups
GT = 512
NT = 16          # 128-token tiles
SCALE = 1.0 / math.sqrt(D_Q)

F32 = mybir.dt.float32
F32R = mybir.dt.float32r
AF = mybir.ActivationFunctionType
ALU = mybir.AluOpType

_CACHE = {}


def _yarn_cos_sin(seq_len):
    # exact numpy replication of the reference cache
    dim = D_ROPE
    ar = np.arange(0, dim, 2, dtype=np.float32)
    freq_extra = 1.0 / BASE ** (ar / dim)
    freq_inter = 1.0 / (FACTOR * BASE ** (ar / dim))

    def corr_dim(num_rot):
        return dim * math.log(ORIG_MAX / (num_rot * 2 * math.pi)) / (2 * math.log(BASE))

    low = max(math.floor(corr_dim(BETA_FAST)), 0)
    high = min(math.ceil(corr_dim(BETA_SLOW)), dim - 1)
    hi = high + 0.001 if low == high else high
    ramp = np.clip((np.arange(dim // 2, dtype=np.float32) - low) / (hi - low), 0.0, 1.0)
    inv_freq_mask = 1.0 - ramp
    inv_freq = freq_inter * (1 - inv_freq_mask) + freq_extra * inv_freq_mask
    t = np.arange(seq_len, dtype=np.float32)
    freqs = np.outer(t, inv_freq)

    def get_mscale(scale, m):
        return 1.0 if scale <= 1 else 0.1 * m * math.log(scale) + 1.0

    msc = float(get_mscale(FACTOR, MSCALE) / get_mscale(FACTOR, MSCALE_ALL))
    emb = np.concatenate([freqs, freqs], axis=-1)
    return (np.cos(emb) * msc).astype(np.float32), (np.sin(emb) * msc).astype(np.float32)


def _to_sbuf_layout(w):
    """(K*128, C) -> (128, K, C) partition-major layout, contiguous."""
    k128, c = w.shape
    return np.ascontiguousarray(w.reshape(k128 // 128, 128, c).transpose(1, 0, 2))


def _build_nc():
    nc = bacc.Bacc("TRN2", target_bir_lowering=False)

    # ---- DRAM I/O (per-core slices supplied via in_maps) ----
    hid_d = nc.dram_tensor("hidT", [128, 16, S], F32R, kind="ExternalInput")
    wnat_d = nc.dram_tensor("wnat", [128, 16, KV_RANK + D_ROPE], F32R, kind="ExternalInput")
    wkvb_d = nc.dram_tensor("wkvb", [128, 4, 1024], F32R, kind="ExternalInput")
    wqn_d = nc.dram_tensor("wqn", [128, 16, 512], F32R, kind="ExternalInput")
    wqr_d = nc.dram_tensor("wqr", [128, 16, 256], F32R, kind="ExternalInput")
    ow_d = nc.dram_tensor("owT", [128, 4, HID], F32R, kind="ExternalInput")
    cos_d = nc.dram_tensor("cosT", [128, 16, D_ROPE], F32, kind="ExternalInput")
    sin_d = nc.dram_tensor("sinT", [128, 16, D_ROPE], F32, kind="ExternalInput")
    y_d = nc.dram_tensor("y", [S, HID], F32, kind="ExternalOutput")
    # internal scratch for attention output (v-major) between phases 2/3
    ot_d = nc.dram_tensor("otmp", [NG, 128, H, GT], F32R, kind="Internal")

    with tile.TileContext(nc) as tc:
        with tc.tile_pool(name="const", bufs=1) as constp, \
             tc.tile_pool(name="persist", bufs=1) as pp, \
             tc.tile_pool(name="hidp", bufs=1) as hidp:

            # constants
            ident_f = constp.tile([128, 128], F32)
            nc.gpsimd.memset(ident_f[:], 0.0)
            nc.gpsimd.affine_select(
                out=ident_f[:], in_=ident_f[:], compare_op=ALU.not_equal,
                fill=1.0, base=0, pattern=[[-1, 128]], channel_multiplier=1)
            ident = constp.tile([128, 128], F32R)
            nc.vector.tensor_copy(ident[:], ident_f[:])
            ones_f = constp.tile([128, 1], F32)
            nc.vector.memset(ones_f[:], 1.0)
            ones = constp.tile([128, 1], F32R)
            nc.vector.tensor_copy(ones[:], ones_f[:])
            eps_t = constp.tile([128, 1], F32)
            nc.vector.memset(eps_t[:], EPS)
            cos_t = constp.tile([128, 16, D_ROPE], F32)
            sin_t = constp.tile([128, 16, D_ROPE], F32)
            nc.sync.dma_start(cos_t[:], cos_d[:])
            nc.sync.dma_start(sin_t[:], sin_d[:])

            # persistent K/V
            knT = pp.tile([128, H, S], F32R)        # k_nope^T per head
            kpeT = pp.tile([64, S], F32R)           # shared roped k_pe^T
            v_all = pp.tile([128, NT, 512], F32R)   # v token-major, 4 heads x 128

            # ---------------- Phase 1: kv path ----------------
            with tc.tile_pool(name="p1w", bufs=1) as wp1, \
                 tc.tile_pool(name="p1s", bufs=2) as sp1, \
                 tc.tile_pool(name="p1ps", bufs=2, space="PSUM") as ps1:
                wnat = wp1.tile([128, 16, KV_RANK + D_ROPE], F32R)
                wkvb = wp1.tile([128, 4, 1024], F32R)
                nc.sync.dma_start(wnat[:], wnat_d[:])
                nc.sync.dma_start(wkvb[:], wkvb_d[:])

                for g in range(NG):
                    hid = hidp.tile([128, 16, GT], F32R, tag="hid")
                    nc.sync.dma_start(hid[:], hid_d[:, :, g * GT:(g + 1) * GT])
                    clnT = sp1.tile([128, 4, GT], F32R, tag="clnT")

                    for t in range(4):
                        tok = slice(t * 128, (t + 1) * 128)
                        gt = g * 4 + t
                        ps_c = ps1.tile([128, 512], F32, tag="c")
                        ps_r = ps1.tile([128, 64], F32, tag="r")
                        for ci in range(16):
                            nc.tensor.matmul(ps_c[:], hid[:, ci, tok], wnat[:, ci, 0:512],
                                             start=(ci == 0), stop=(ci == 15))
                            nc.tensor.matmul(ps_r[:], hid[:, ci, tok], wnat[:, ci, 512:576],
                                             start=(ci == 0), stop=(ci == 15))
                        # RMSNorm over free axis
                        ssum = sp1.tile([128, 1], F32, tag="ssum")
                        sq = sp1.tile([128, 512], F32, tag="sq")
                        nc.scalar.activation(sq[:], ps_c[:], AF.Square, accum_out=ssum[:])
                        rstd = sp1.tile([128, 1], F32, tag="rstd")
                        nc.scalar.activation(rstd[:], ssum[:], AF.Sqrt,
                                             scale=1.0 / KV_RANK, bias=eps_t[:])
                        rinv = sp1.tile([128, 1], F32, tag="rinv")
                        nc.vector.reciprocal(rinv[:], rstd[:])
                        c_ln = sp1.tile([128, 512], F32R, tag="cln")
                        nc.vector.tensor_scalar_mul(c_ln[:], ps_c[:], rinv[:])
                        # RoPE on k_pe (token-major)
                        cs = cos_t[:, gt, :]
                        sn = sin_t[:, gt, :]
                        kt1 = sp1.tile([128, 64], F32, tag="kt1")
                        kt2 = sp1.tile([128, 64], F32, tag="kt2")
                        nc.vector.tensor_mul(kt1[:], ps_r[:], cs)
                        nc.vector.tensor_mul(kt2[:, 0:32], ps_r[:, 32:64], sn[:, 0:32])
                        nc.vector.tensor_mul(kt2[:, 32:64], ps_r[:, 0:32], sn[:, 32:64])
                        kpe_nat = sp1.tile([128, 64], F32R, tag="kpenat")
                        nc.vector.tensor_add(kpe_nat[:], kt1[:], kt2[:])
                        # transposes: c_ln (4x128) and k_pe (64)
                        for cc in range(4):
                            pst = ps1.tile([128, 128], F32R, tag="tr")
                            nc.tensor.transpose(pst[:], c_ln[:, cc * 128:(cc + 1) * 128], ident[:])
                            nc.vector.tensor_copy(clnT[:, cc, tok], pst[:])
                        pst = ps1.tile([128, 128], F32R, tag="tr")
                        nc.tensor.transpose(pst[0:64, :], kpe_nat[:], ident[:])
                        nc.vector.tensor_copy(kpeT[:, gt * 128:(gt + 1) * 128], pst[0:64, :])

                    # kv_b decompression for the group
                    for h in range(H):
                        ps_k = ps1.tile([128, 512], F32, tag="kv")
                        for cc in range(4):
                            nc.tensor.matmul(ps_k[:], wkvb[:, cc, h * 128:(h + 1) * 128],
                                             clnT[:, cc, :], start=(cc == 0), stop=(cc == 3))
                        nc.vector.tensor_copy(knT[:, h, g * GT:(g + 1) * GT], ps_k[:])
                    for t in range(4):
                        ps_v = ps1.tile([128, 512], F32, tag="kv")
                        for cc in range(4):
                            nc.tensor.matmul(ps_v[:], clnT[:, cc, t * 128:(t + 1) * 128],
                                             wkvb[:, cc, 512:1024], start=(cc == 0), stop=(cc == 3))
                        nc.vector.tensor_copy(v_all[:, g * 4 + t, :], ps_v[:])

            # ---------------- Phase 2: q + attention ----------------
            with tc.tile_pool(name="p2w", bufs=1) as wp2, \
                 tc.tile_pool(name="p2s", bufs=2) as sp2, \
                 tc.tile_pool(name="p2q", bufs=1) as qp, \
                 tc.tile_pool(name="p2pt", bufs=3) as ptp, \
                 tc.tile_pool(name="p2ps", bufs=1, space="PSUM") as ps2:
                wqn = wp2.tile([128, 16, 512], F32R)
                wqr = wp2.tile([128, 16, 256], F32R)
                nc.sync.dma_start(wqn[:], wqn_d[:])
                nc.sync.dma_start(wqr[:], wqr_d[:])

                for qi in range(NG):
                    hid = hidp.tile([128, 16, GT], F32R, tag="hid")
                    nc.sync.dma_start(hid[:], hid_d[:, :, qi * GT:(qi + 1) * GT])

                    # q_nope^T directly (weights stationary), scaled
                    qnT = qp.tile([128, H, GT], F32R, tag="qn")
                    for h in range(H):
                        ps_q = ps2.tile([128, 512], F32, tag="mm512a")
                        for ci in range(16):
                            nc.tensor.matmul(ps_q[:], wqn[:, ci, h * 128:(h + 1) * 128],
                                             hid[:, ci, :], start=(ci == 0), stop=(ci == 15))
                        nc.vector.tensor_scalar_mul(qnT[:, h, :], ps_q[:], SCALE)

                    # q_rope: token-major proj, RoPE, transpose, scale
                    qrT = qp.tile([64, H, GT], F32R, tag="qr")
                    for t in range(4):
                        tok = slice(t * 128, (t + 1) * 128)
                        gt = qi * 4 + t
                        ps_r = ps2.tile([128, 256], F32, tag="mm512b")
                        for ci in range(16):
                            nc.tensor.matmul(ps_r[:], hid[:, ci, tok], wqr[:, ci, :],
                                             start=(ci == 0), stop=(ci == 15))
                        cs = cos_t[:, gt, :]
                        sn = sin_t[:, gt, :]
                        for h in range(H):
                            rs = ps_r[:, h * 64:(h + 1) * 64]
                            t1 = sp2.tile([128, 64], F32, tag="qt1")
                            t2 = sp2.tile([128, 64], F32, tag="qt2")
                            nc.vector.tensor_mul(t1[:], rs, cs)
                            nc.vector.tensor_mul(t2[:, 0:32], rs[:, 32:64], sn[:, 0:32])
                            nc.vector.tensor_mul(t2[:, 32:64], rs[:, 0:32], sn[:, 32:64])
                            qnat = sp2.tile([128, 64], F32R, tag="qnat")
                            nc.vector.tensor_add(qnat[:], t1[:], t2[:])
                            pst = ps2.tile([128, 128], F32R, tag="tr")
                            nc.tensor.transpose(pst[0:64, :], qnat[:], ident[:])
                            nc.vector.tensor_scalar_mul(qrT[:, h, tok], pst[0:64, :], SCALE)

                    # attention for this q group
                    nkc = (qi + 1) * 4
                    for h in range(H):
                        ps_l = ps2.tile([1, 512], F32, tag="l")
                        ps_o = ps2.tile([128, 512], F32, tag="o")
                        for kc in range(nkc):
                            ks = slice(kc * 128, (kc + 1) * 128)
                            ps_s = ps2.tile([128, 512], F32, tag="s")
                            nc.tensor.matmul(ps_s[:], knT[:, h, ks], qnT[:, h, :],
                                             start=True, stop=False)
                            nc.tensor.matmul(ps_s[:], kpeT[:, ks], qrT[:, h, :],
                                             start=False, stop=True)
                            pT = ptp.tile([128, 512], F32R, tag="pT")
                            nc.scalar.activation(pT[:], ps_s[:], AF.Exp)
                            if kc >= nkc - 4:
                                # keep where k_abs <= q_abs:
                                # -p + f + (qi*512 - kc*128) >= 0
                                nc.gpsimd.affine_select(
                                    out=pT[:], in_=pT[:], compare_op=ALU.is_ge,
                                    fill=0.0, base=qi * 512 - kc * 128,
                                    pattern=[[1, 512]], channel_multiplier=-1)
                            nc.tensor.matmul(ps_l[:], ones[:], pT[:],
                                             start=(kc == 0), stop=(kc == nkc - 1))
                            nc.tensor.matmul(ps_o[:], v_all[:, kc, h * 128:(h + 1) * 128],
                                             pT[:], start=(kc == 0), stop=(kc == nkc - 1))
                        linv = sp2.tile([1, 512], F32, tag="linv")
                        nc.vector.reciprocal(linv[:], ps_l[:])
                        linvb = sp2.tile([128, 512], F32, tag="linvb")
                        nc.gpsimd.partition_broadcast(linvb[:], linv[:])
                        oT = sp2.tile([128, 512], F32R, tag="oT")
                        nc.vector.tensor_mul(oT[:], ps_o[:], linvb[:])
                        nc.sync.dma_start(ot_d[qi, :, h, :], oT[:])

            # ---------------- Phase 3: o-projection ----------------
            with tc.tile_pool(name="p3w", bufs=1) as wp3, \
                 tc.tile_pool(name="p3s", bufs=2) as sp3, \
                 tc.tile_pool(name="p3ps", bufs=2, space="PSUM") as ps3:
                ow = wp3.tile([128, 4, HID], F32R)
                nc.sync.dma_start(ow[:], ow_d[:])
                for qi in range(NG):
                    oTl = sp3.tile([128, H, GT], F32R, tag="oTl")
                    nc.sync.dma_start(oTl[:], ot_d[qi])
                    for qs in range(4):
                        qabs = qi * GT + qs * 128
                        for hoc in range(4):
                            ps_y = ps3.tile([128, 512], F32, tag="y")
                            for h in range(H):
                                nc.tensor.matmul(
                                    ps_y[:], oTl[:, h, qs * 128:(qs + 1) * 128],
                                    ow[:, h, hoc * 512:(hoc + 1) * 512],
                                    start=(h == 0), stop=(h == 3))
                            ysb = sp3.tile([128, 512], F32, tag="ysb")
                            nc.vector.tensor_copy(ysb[:], ps_y[:])
                            nc.sync.dma_start(
                                y_d[qabs:qabs + 128, hoc * 512:(hoc + 1) * 512], ysb[:])
    nc.compile()
    return nc


_DEINT_PERM = np.concatenate([np.arange(0, 64, 2), np.arange(1, 64, 2)])


def _prep_inputs(hidden_states, position_ids, q_w, kv_a_w, kv_a_ln_w, kv_b_w, o_w):
    """Build the 8 per-core input maps (host-side slicing/layout only)."""
    hidden_states = np.asarray(hidden_states, dtype=np.float32)
    position_ids = np.asarray(position_ids)
    q_w = np.asarray(q_w, dtype=np.float32)
    kv_a_w = np.asarray(kv_a_w, dtype=np.float32)
    kv_a_ln_w = np.asarray(kv_a_ln_w, dtype=np.float32)
    kv_b_w = np.asarray(kv_b_w, dtype=np.float32)
    o_w = np.asarray(o_w, dtype=np.float32)

    cos_c, sin_c = _yarn_cos_sin(MAX_POS)

    # per-batch tensors
    hidT = []
    cos_b, sin_b = [], []
    for b in range(B):
        hidT.append(_to_sbuf_layout(np.ascontiguousarray(hidden_states[b].T)))
        pos = position_ids[b].astype(np.int64)
        c = cos_c[pos]                     # (S, 64)
        s = sin_c[pos]
        s_eff = np.concatenate([-s[:, :32], s[:, 32:]], axis=1)
        cos_b.append(_to_sbuf_layout(c))
        sin_b.append(_to_sbuf_layout(s_eff))

    # kv_a weights: c rows + deinterleaved k_pe rows; transposed layout
    kpe_rows = kv_a_w[KV_RANK + _DEINT_PERM]           # (64, HID)
    wnat = np.concatenate([kv_a_w[:KV_RANK], kpe_rows], axis=0)  # (576, HID)
    wnat_l = _to_sbuf_layout(np.ascontiguousarray(wnat.T))        # (128,16,576)

    in_maps = []
    for core in range(8):
        b, hg = core // 4, core % 4
        heads = range(4 * hg, 4 * hg + 4)
        # q weights
        wqn = np.concatenate([q_w[h * D_Q: h * D_Q + D_NOPE] for h in heads], axis=0)
        wqr = np.concatenate(
            [q_w[h * D_Q + D_NOPE + _DEINT_PERM] for h in heads], axis=0)
        wqn_l = _to_sbuf_layout(np.ascontiguousarray(wqn.T))      # (128,16,512)
        wqr_l = _to_sbuf_layout(np.ascontiguousarray(wqr.T))      # (128,16,256)
        # kv_b weights with ln absorbed: [k rows (4x128) | v rows (4x128)]
        wk = np.concatenate(
            [kv_b_w[h * (D_NOPE + D_V): h * (D_NOPE + D_V) + D_NOPE] for h in heads],
            axis=0) * kv_a_ln_w[None, :]
        wv = np.concatenate(
            [kv_b_w[h * (D_NOPE + D_V) + D_NOPE: (h + 1) * (D_NOPE + D_V)] for h in heads],
            axis=0) * kv_a_ln_w[None, :]
        wkvb = np.concatenate([wk, wv], axis=0)                   # (1024, 512)
        wkvb_l = _to_sbuf_layout(np.ascontiguousarray(wkvb.T))    # (128,4,1024)
        # o_w slice (input-dim shard)
        owT = np.ascontiguousarray(o_w[:, 4 * hg * D_V: (4 * hg + 4) * D_V].T)  # (512, HID)
        ow_l = _to_sbuf_layout(owT)                               # (128,4,2048)

        in_maps.append({
            "hidT": hidT[b],
            "wnat": wnat_l,
            "wkvb": wkvb_l,
            "wqn": wqn_l,
            "wqr": wqr_l,
            "owT": ow_l,
            "cosT": cos_b[b],
            "sinT": sin_b[b],
        })
    return in_maps


last_results = None


def kernel(hidden_states, position_ids, q_w, kv_a_w, kv_a_ln_w, kv_b_w, o_w):
    global last_results
    if "nc" not in _CACHE:
        _CACHE["nc"] = _build_nc()
    nc = _CACHE["nc"]
    in_maps = _prep_inputs(hidden_states, position_ids, q_w, kv_a_w,
                           kv_a_ln_w, kv_b_w, o_w)
    res = run_bass_kernel_spmd(nc, in_maps, core_ids=list(range(8)))
    last_results = res
    out = np.zeros((B, S, HID), dtype=np.float32)
    for core in range(8):
        out[core // 4] += res.results[core]["y"]
    return out


# revision 8
# speedup vs baseline: 1.2317x; 1.2317x over previous
"""DeepseekV2-Lite MLA attention on 8 Trainium2 NeuronCores.

Sharding: DP2 (batch) x TP4 (4 heads/core). Core c handles batch c//4,
heads [4*(c%4), 4*(c%4)+4). Each core computes a partial output (its
heads' contribution to the o-projection); the host sums 4 partials per
batch.

Device algorithm per core (T=2048 tokens, H=4 heads), all big matmuls in
float32r (TF32-like, full PE rate at N>=256):
  Phase 1 (per 512-token group): kv_a projection (c + shared k_pe) in
    token-major layout, RMSNorm over free axis, RoPE on k_pe, PE-transpose
    c_ln -> c_lnT, kv_b decompression to k_nopeT (head-major transposed)
    and v (token-major).
  Phase 2 (per 512-q group): q projections (nope directly transposed,
    rope token-major then RoPE + PE-transpose), then flash-style
    attention with k-major score tiles: sT = kT q, p = exp(sT) (no
    max subtraction -- |scores| <= ~5), causal mask via affine_select,
    l = ones^T p via matmul, outT = v^T p accumulated in PSUM, out/l.
  Phase 3: o-projection from outT (DRAM-staged) with per-core o_w slice.
"""

import math
import numpy as np

import concourse.bacc as bacc
import concourse.mybir as mybir
import concourse.tile as tile
from concourse.bass_utils import run_bass_kernel_spmd

# ---- problem constants (hardcoded per contest contract) ----
B, S, HID = 2, 2048, 2048
NH, D_NOPE, D_ROPE, D_V = 16, 128, 64, 128
D_Q = D_NOPE + D_ROPE
KV_RANK = 512
EPS = 1e-6
MAX_POS = 8192
BASE = 10000.0
FACTOR = 40.0
ORIG_MAX = 4096
BETA_FAST, BETA_SLOW = 32, 1
MSCALE, MSCALE_ALL = 0.707, 0.707

H = 4            # heads per core
NG = 4           # 512-token groups
GT = 512
NT = 16          # 128-token tiles
SCALE = 1.0 / math.sqrt(D_Q)

F32 = mybir.dt.float32
F32R = mybir.dt.float32r
AF = mybir.ActivationFunctionType
ALU = mybir.AluOpType

_CACHE = {}


def _yarn_cos_sin(seq_len):
    # exact numpy replication of the reference cache
    dim = D_ROPE
    ar = np.arange(0, dim, 2, dtype=np.float32)
    freq_extra = 1.0 / BASE ** (ar / dim)
    freq_inter = 1.0 / (FACTOR * BASE ** (ar / dim))

    def corr_dim(num_rot):
        return dim * math.log(ORIG_MAX / (num_rot * 2 * math.pi)) / (2 * math.log(BASE))

    low = max(math.floor(corr_dim(BETA_FAST)), 0)
    high = min(math.ceil(corr_dim(BETA_SLOW)), dim - 1)
    hi = high + 0.001 if low == high else high
    ramp = np.clip((np.arange(dim // 2, dtype=np.float32) - low) / (hi - low), 0.0, 1.0)
    inv_freq_mask = 1.0 - ramp
    inv_freq = freq_inter * (1 - inv_freq_mask) + freq_extra * inv_freq_mask
    t = np.arange(seq_len, dtype=np.float32)
    freqs = np.outer(t, inv_freq)

    def get_mscale(scale, m):
        return 1.0 if scale <= 1 else 0.1 * m * math.log(scale) + 1.0

    msc = float(get_mscale(FACTOR, MSCALE) / get_mscale(FACTOR, MSCALE_ALL))
    emb = np.concatenate([freqs, freqs], axis=-1)
    return (np.cos(emb) * msc).astype(np.float32), (np.sin(emb) * msc).astype(np.float32)


def _to_sbuf_layout(w):
    """(K*128, C) -> (128, K, C) partition-major layout, contiguous."""
    k128, c = w.shape
    return np.ascontiguousarray(w.reshape(k128 // 128, 128, c).transpose(1, 0, 2))


def _build_nc():
    nc = bacc.Bacc("TRN2", target_bir_lowering=False)

    # ---- DRAM I/O (per-core slices supplied via in_maps) ----
    hid_d = nc.dram_tensor("hidT", [128, 16, S], F32R, kind="ExternalInput")
    wnat_d = nc.dram_tensor("wnat", [128, 16, KV_RANK + D_ROPE], F32R, kind="ExternalInput")
    wkvb_d = nc.dram_tensor("wkvb", [128, 4, 1024], F32R, kind="ExternalInput")
    wqn_d = nc.dram_tensor("wqn", [128, 16, 512], F32R, kind="ExternalInput")
    wqr_d = nc.dram_tensor("wqr", [128, 16, 256], F32R, kind="ExternalInput")
    ow_d = nc.dram_tensor("owT", [128, 4, HID], F32R, kind="ExternalInput")
    cos_d = nc.dram_tensor("cosT", [128, 16, D_ROPE], F32, kind="ExternalInput")
    sin_d = nc.dram_tensor("sinT", [128, 16, D_ROPE], F32, kind="ExternalInput")
    y_d = nc.dram_tensor("y", [S, HID], F32, kind="ExternalOutput")
    # internal scratch for attention output (v-major) between phases 2/3
    ot_d = nc.dram_tensor("otmp", [NG, 128, H, GT], F32R, kind="Internal")

    with tile.TileContext(nc) as tc:
        with tc.tile_pool(name="const", bufs=1) as constp, \
             tc.tile_pool(name="persist", bufs=1) as pp, \
             tc.tile_pool(name="hidp", bufs=1) as hidp:

            # constants
            ident_f = constp.tile([128, 128], F32)
            nc.gpsimd.memset(ident_f[:], 0.0)
            nc.gpsimd.affine_select(
                out=ident_f[:], in_=ident_f[:], compare_op=ALU.not_equal,
                fill=1.0, base=0, pattern=[[-1, 128]], channel_multiplier=1)
            ident = constp.tile([128, 128], F32R)
            nc.vector.tensor_copy(ident[:], ident_f[:])
            ones_f = constp.tile([128, 1], F32)
            nc.vector.memset(ones_f[:], 1.0)
            ones = constp.tile([128, 1], F32R)
            nc.vector.tensor_copy(ones[:], ones_f[:])
            eps_t = constp.tile([128, 1], F32)
            nc.vector.memset(eps_t[:], EPS)
            masks = constp.tile([128, 4, 512], F32R)
            with tc.tile_pool(name="mtmp", bufs=1) as mtp:
                mask_f = mtp.tile([128, 4, 512], F32)
                nc.gpsimd.memset(mask_f[:], 1.0)
                for j in range(4):
                    # keep where f - p - j*128 >= 0 (k_abs <= q_abs on diagonal)
                    nc.gpsimd.affine_select(
                        out=mask_f[:, j, :], in_=mask_f[:, j, :], compare_op=ALU.is_ge,
                        fill=0.0, base=-j * 128, pattern=[[1, 512]], channel_multiplier=-1)
                nc.vector.tensor_copy(masks[:], mask_f[:])
            cos_t = constp.tile([128, 16, D_ROPE], F32)
            sin_t = constp.tile([128, 16, D_ROPE], F32)
            nc.sync.dma_start(cos_t[:], cos_d[:])
            nc.sync.dma_start(sin_t[:], sin_d[:])

            # persistent K/V
            knT = pp.tile([128, H, S], F32R)        # k_nope^T per head
            kpeT = pp.tile([64, S], F32R)           # shared roped k_pe^T
            v_all = pp.tile([128, NT, 512], F32R)   # v token-major, 4 heads x 128

            # ---------------- Phase 1: kv path ----------------
            with tc.tile_pool(name="p1w", bufs=1) as wp1, \
                 tc.tile_pool(name="p1s", bufs=2) as sp1, \
                 tc.tile_pool(name="p1ps", bufs=2, space="PSUM") as ps1:
                wnat = wp1.tile([128, 16, KV_RANK + D_ROPE], F32R)
                wkvb = wp1.tile([128, 4, 1024], F32R)
                nc.sync.dma_start(wnat[:], wnat_d[:])
                nc.sync.dma_start(wkvb[:], wkvb_d[:])

                for g in range(NG):
                    hid = hidp.tile([128, 16, GT], F32R, tag="hid")
                    nc.sync.dma_start(hid[:], hid_d[:, :, g * GT:(g + 1) * GT])
                    clnT = sp1.tile([128, 4, GT], F32R, tag="clnT")

                    for t in range(4):
                        tok = slice(t * 128, (t + 1) * 128)
                        gt = g * 4 + t
                        ps_c = ps1.tile([128, 512], F32, tag="c")
                        ps_r = ps1.tile([128, 64], F32, tag="r")
                        for ci in range(16):
                            nc.tensor.matmul(ps_c[:], hid[:, ci, tok], wnat[:, ci, 0:512],
                                             start=(ci == 0), stop=(ci == 15))
                            nc.tensor.matmul(ps_r[:], hid[:, ci, tok], wnat[:, ci, 512:576],
                                             start=(ci == 0), stop=(ci == 15))
                        # RMSNorm over free axis
                        ssum = sp1.tile([128, 1], F32, tag="ssum")
                        sq = sp1.tile([128, 512], F32, tag="sq")
                        nc.scalar.activation(sq[:], ps_c[:], AF.Square, accum_out=ssum[:])
                        rstd = sp1.tile([128, 1], F32, tag="rstd")
                        nc.scalar.activation(rstd[:], ssum[:], AF.Sqrt,
                                             scale=1.0 / KV_RANK, bias=eps_t[:])
                        rinv = sp1.tile([128, 1], F32, tag="rinv")
                        nc.vector.reciprocal(rinv[:], rstd[:])
                        c_ln = sp1.tile([128, 512], F32R, tag="cln")
                        nc.vector.tensor_scalar_mul(c_ln[:], ps_c[:], rinv[:])
                        # RoPE on k_pe (token-major)
                        cs = cos_t[:, gt, :]
                        sn = sin_t[:, gt, :]
                        kt1 = sp1.tile([128, 64], F32, tag="kt1")
                        kt2 = sp1.tile([128, 64], F32, tag="kt2")
                        nc.vector.tensor_mul(kt1[:], ps_r[:], cs)
                        nc.vector.tensor_mul(kt2[:, 0:32], ps_r[:, 32:64], sn[:, 0:32])
                        nc.vector.tensor_mul(kt2[:, 32:64], ps_r[:, 0:32], sn[:, 32:64])
                        kpe_nat = sp1.tile([128, 64], F32R, tag="kpenat")
                        nc.vector.tensor_add(kpe_nat[:], kt1[:], kt2[:])
                        # transposes: c_ln (4x128) and k_pe (64)
                        for cc in range(4):
                            pst = ps1.tile([128, 128], F32R, tag="tr")
                            nc.tensor.transpose(pst[:], c_ln[:, cc * 128:(cc + 1) * 128], ident[:])
                            nc.vector.tensor_copy(clnT[:, cc, tok], pst[:])
                        pst = ps1.tile([128, 128], F32R, tag="tr")
                        nc.tensor.transpose(pst[0:64, :], kpe_nat[:], ident[:])
                        nc.vector.tensor_copy(kpeT[:, gt * 128:(gt + 1) * 128], pst[0:64, :])

                    # kv_b decompression for the group
                    for h in range(H):
                        ps_k = ps1.tile([128, 512], F32, tag="kv")
                        for cc in range(4):
                            nc.tensor.matmul(ps_k[:], wkvb[:, cc, h * 128:(h + 1) * 128],
                                             clnT[:, cc, :], start=(cc == 0), stop=(cc == 3))
                        nc.vector.tensor_copy(knT[:, h, g * GT:(g + 1) * GT], ps_k[:])
                    for t in range(4):
                        ps_v = ps1.tile([128, 512], F32, tag="kv")
                        for cc in range(4):
                            nc.tensor.matmul(ps_v[:], clnT[:, cc, t * 128:(t + 1) * 128],
                                             wkvb[:, cc, 512:1024], start=(cc == 0), stop=(cc == 3))
                        nc.vector.tensor_copy(v_all[:, g * 4 + t, :], ps_v[:])

            # ---------------- Phase 2: q + attention ----------------
            with tc.tile_pool(name="p2w", bufs=1) as wp2, \
                 tc.tile_pool(name="p2s", bufs=2) as sp2, \
                 tc.tile_pool(name="p2sb", bufs=1) as sp2b, \
                 tc.tile_pool(name="p2q", bufs=1) as qp, \
                 tc.tile_pool(name="p2pt", bufs=2) as ptp, \
                 tc.tile_pool(name="p2ps", bufs=2, space="PSUM") as ps2, \
                 tc.tile_pool(name="p2pq", bufs=2, space="PSUM") as ps2q:
                wqn = wp2.tile([128, 16, 512], F32R)
                wqr = wp2.tile([128, 16, 256], F32R)
                nc.sync.dma_start(wqn[:], wqn_d[:])
                nc.sync.dma_start(wqr[:], wqr_d[:])

                for qi in range(NG):
                    hid = hidp.tile([128, 16, GT], F32R, tag="hid")
                    nc.sync.dma_start(hid[:], hid_d[:, :, qi * GT:(qi + 1) * GT])

                    # q_nope^T directly (weights stationary), scaled
                    qnT = qp.tile([128, H, GT], F32R, tag="qn")
                    for h in range(H):
                        ps_q = ps2q.tile([128, 512], F32, tag="qmm")
                        for ci in range(16):
                            nc.tensor.matmul(ps_q[:], wqn[:, ci, h * 128:(h + 1) * 128],
                                             hid[:, ci, :], start=(ci == 0), stop=(ci == 15))
                        nc.vector.tensor_scalar_mul(qnT[:, h, :], ps_q[:], SCALE)

                    # q_rope: token-major proj, RoPE, transpose, scale
                    qrT = qp.tile([64, H, GT], F32R, tag="qr")
                    for t in range(4):
                        tok = slice(t * 128, (t + 1) * 128)
                        gt = qi * 4 + t
                        ps_r = ps2q.tile([128, 512], F32, tag="qmm")
                        for ci in range(16):
                            nc.tensor.matmul(ps_r[:, 0:256], hid[:, ci, tok], wqr[:, ci, :],
                                             start=(ci == 0), stop=(ci == 15))
                        cs = cos_t[:, gt, :]
                        sn = sin_t[:, gt, :]
                        for h in range(H):
                            rs = ps_r[:, h * 64:(h + 1) * 64]
                            t1 = sp2.tile([128, 64], F32, tag="qt1")
                            t2 = sp2.tile([128, 64], F32, tag="qt2")
                            nc.vector.tensor_mul(t1[:], rs, cs)
                            nc.vector.tensor_mul(t2[:, 0:32], rs[:, 32:64], sn[:, 0:32])
                            nc.vector.tensor_mul(t2[:, 32:64], rs[:, 0:32], sn[:, 32:64])
                            qnat = sp2.tile([128, 64], F32R, tag="qnat")
                            nc.vector.tensor_add(qnat[:], t1[:], t2[:])
                            pst = ps2q.tile([128, 128], F32R, tag="qmm")
                            nc.tensor.transpose(pst[0:64, :], qnat[:], ident[:])
                            nc.vector.tensor_scalar_mul(qrT[:, h, tok], pst[0:64, :], SCALE)

                    # attention for this q group
                    nkc = (qi + 1) * 4
                    for h in range(H):
                        ps_l = ps2.tile([1, 512], F32, tag="l")
                        ps_o = ps2.tile([128, 512], F32, tag="o")
                        for kc in range(nkc):
                            ks = slice(kc * 128, (kc + 1) * 128)
                            ps_s = ps2.tile([128, 512], F32, tag="s")
                            nc.tensor.matmul(ps_s[:], knT[:, h, ks], qnT[:, h, :],
                                             start=True, stop=False)
                            nc.tensor.matmul(ps_s[:], kpeT[:, ks], qrT[:, h, :],
                                             start=False, stop=True)
                            pT = ptp.tile([128, 512], F32R, tag="pT")
                            nc.scalar.activation(pT[:], ps_s[:], AF.Exp)
                            if kc >= nkc - 4:
                                j = kc - qi * 4
                                nc.vector.tensor_mul(pT[:], pT[:], masks[:, j, :])
                            nc.tensor.matmul(ps_l[:], ones[:], pT[:],
                                             start=(kc == 0), stop=(kc == nkc - 1))
                            nc.tensor.matmul(ps_o[:], v_all[:, kc, h * 128:(h + 1) * 128],
                                             pT[:], start=(kc == 0), stop=(kc == nkc - 1))
                        linv = sp2.tile([1, 512], F32, tag="linv")
                        nc.vector.reciprocal(linv[:], ps_l[:])
                        linvb = sp2b.tile([128, 512], F32, tag="linvb")
                        nc.gpsimd.partition_broadcast(linvb[:], linv[:])
                        oT = sp2b.tile([128, 512], F32R, tag="oT")
                        nc.vector.tensor_mul(oT[:], ps_o[:], linvb[:])
                        nc.sync.dma_start(ot_d[qi, :, h, :], oT[:])

            # ---------------- Phase 3: o-projection ----------------
            with tc.tile_pool(name="p3w", bufs=1) as wp3, \
                 tc.tile_pool(name="p3s", bufs=2) as sp3, \
                 tc.tile_pool(name="p3ps", bufs=2, space="PSUM") as ps3:
                ow = wp3.tile([128, 4, HID], F32R)
                nc.sync.dma_start(ow[:], ow_d[:])
                for qi in range(NG):
                    oTl = sp3.tile([128, H, GT], F32R, tag="oTl")
                    nc.sync.dma_start(oTl[:], ot_d[qi])
                    for qs in range(4):
                        qabs = qi * GT + qs * 128
                        for hoc in range(4):
                            ps_y = ps3.tile([128, 512], F32, tag="y")
                            for h in range(H):
                                nc.tensor.matmul(
                                    ps_y[:], oTl[:, h, qs * 128:(qs + 1) * 128],
                                    ow[:, h, hoc * 512:(hoc + 1) * 512],
                                    start=(h == 0), stop=(h == 3))
                            ysb = sp3.tile([128, 512], F32, tag="ysb")
                            nc.vector.tensor_copy(ysb[:], ps_y[:])
                            nc.sync.dma_start(
                                y_d[qabs:qabs + 128, hoc * 512:(hoc + 1) * 512], ysb[:])
    nc.compile()
    return nc


_DEINT_PERM = np.concatenate([np.arange(0, 64, 2), np.arange(1, 64, 2)])


def _prep_inputs(hidden_states, position_ids, q_w, kv_a_w, kv_a_ln_w, kv_b_w, o_w):
    """Build the 8 per-core input maps (host-side slicing/layout only)."""
    hidden_states = np.asarray(hidden_states, dtype=np.float32)
    position_ids = np.asarray(position_ids)
    q_w = np.asarray(q_w, dtype=np.float32)
    kv_a_w = np.asarray(kv_a_w, dtype=np.float32)
    kv_a_ln_w = np.asarray(kv_a_ln_w, dtype=np.float32)
    kv_b_w = np.asarray(kv_b_w, dtype=np.float32)
    o_w = np.asarray(o_w, dtype=np.float32)

    cos_c, sin_c = _yarn_cos_sin(MAX_POS)

    # per-batch tensors
    hidT = []
    cos_b, sin_b = [], []
    for b in range(B):
        hidT.append(_to_sbuf_layout(np.ascontiguousarray(hidden_states[b].T)))
        pos = position_ids[b].astype(np.int64)
        c = cos_c[pos]                     # (S, 64)
        s = sin_c[pos]
        s_eff = np.concatenate([-s[:, :32], s[:, 32:]], axis=1)
        cos_b.append(_to_sbuf_layout(c))
        sin_b.append(_to_sbuf_layout(s_eff))

    # kv_a weights: c rows + deinterleaved k_pe rows; transposed layout
    kpe_rows = kv_a_w[KV_RANK + _DEINT_PERM]           # (64, HID)
    wnat = np.concatenate([kv_a_w[:KV_RANK], kpe_rows], axis=0)  # (576, HID)
    wnat_l = _to_sbuf_layout(np.ascontiguousarray(wnat.T))        # (128,16,576)

    in_maps = []
    for core in range(8):
        b, hg = core // 4, core % 4
        heads = range(4 * hg, 4 * hg + 4)
        # q weights
        wqn = np.concatenate([q_w[h * D_Q: h * D_Q + D_NOPE] for h in heads], axis=0)
        wqr = np.concatenate(
            [q_w[h * D_Q + D_NOPE + _DEINT_PERM] for h in heads], axis=0)
        wqn_l = _to_sbuf_layout(np.ascontiguousarray(wqn.T))      # (128,16,512)
        wqr_l = _to_sbuf_layout(np.ascontiguousarray(wqr.T))      # (128,16,256)
        # kv_b weights with ln absorbed: [k rows (4x128) | v rows (4x128)]
        wk = np.concatenate(
            [kv_b_w[h * (D_NOPE + D_V): h * (D_NOPE + D_V) + D_NOPE] for h in heads],
            axis=0) * kv_a_ln_w[None, :]
        wv = np.concatenate(
            [kv_b_w[h * (D_NOPE + D_V) + D_NOPE: (h + 1) * (D_NOPE + D_V)] for h in heads],
            axis=0) * kv_a_ln_w[None, :]
        wkvb = np.concatenate([wk, wv], axis=0)                   # (1024, 512)
        wkvb_l = _to_sbuf_layout(np.ascontiguousarray(wkvb.T))    # (128,4,1024)
        # o_w slice (input-dim shard)
        owT = np.ascontiguousarray(o_w[:, 4 * hg * D_V: (4 * hg + 4) * D_V].T)  # (512, HID)
        ow_l = _to_sbuf_layout(owT)                               # (128,4,2048)

        in_maps.append({
            "hidT": hidT[b],
            "wnat": wnat_l,
            "wkvb": wkvb_l,
            "wqn": wqn_l,
            "wqr": wqr_l,
            "owT": ow_l,
            "cosT": cos_b[b],
            "sinT": sin_b[b],
        })
    return in_maps


last_results = None


def kernel(hidden_states, position_ids, q_w, kv_a_w, kv_a_ln_w, kv_b_w, o_w):
    global last_results
    if "nc" not in _CACHE:
        _CACHE["nc"] = _build_nc()
    nc = _CACHE["nc"]
    in_maps = _prep_inputs(hidden_states, position_ids, q_w, kv_a_w,
                           kv_a_ln_w, kv_b_w, o_w)
    res = run_bass_kernel_spmd(nc, in_maps, core_ids=list(range(8)))
    last_results = res
    out = np.zeros((B, S, HID), dtype=np.float32)
    for core in range(8):
        out[core // 4] += res.results[core]["y"]
    return out
